# revision 25
# baseline (speedup 1.0000x reference)
"""MinGRU Trainium2 kernel.

Math (linear-space reformulation of the reference's log-space scan; all
quantities are positive so this is numerically safe):
    k = x @ W_z.T ; u = x @ W_h.T
    c_t = sigmoid(-k_t)                  # decay coeff (1 - z_t)
    g_t = max(sigmoid(u_t), u_t + 0.5)   # == relu(u) + sigmoid(min(u,0))
    vt_t = (c_t - 1) * g_t               # = -(z_t * g_t)
    h_t = c_t * h_{t-1} - vt_t           # hardware tensor_tensor_scan (mult, subtract)
    h_0 = g(h0)

Sharding: 8 cores = 4 batches x 2 halves of d_model output channels.
Each core: x.T [1024,4096] fp16, weight slices W.T[:, half] [1024,512]
fp16, out h.T [512,4096] f32.  fp16 upconverts exactly into the PE's
internal format and 2-byte stationary weights take the fast-weight-load
path so LDWEIGHTS (~97ns) hides under the previous matmul.

DMA layouts are host-packed so chunk transfers are contiguous per SBUF
partition (128 descriptors of 8-16KB), cutting DMA descriptor processing
~4x.  Weights are packed per e-tile so the first matmul group only waits
on 1/4 of W_z.  The scan keeps fp32 output (fp16 scan output hits a DVE
slow path measured at 3.5x).  Issue queues: sync carries wz/wh + x chunk
loads, scalar carries x0 + h writes; tail e-tile writes alternate
sync/scalar.  Dummy matmuls at t=0 (gated only on a gpsimd memset) keep
the PE busy/ramping through the input DMA ramp.
"""

import numpy as np

B, T, D = 4, 4096, 1024
EC = 512            # output channels per core
ET = EC // 128      # 4 e-tiles per core
KT = D // 128       # 8 k-tiles
SIZES = [256] + [512] * 7 + [256]   # time chunks (sum = T)
N_TAIL = 1                          # last chunk uses the low-latency tail path

_CACHED = {}
LAST_RESULT = None


def _build_nc():
    import concourse.bass as bass
    import concourse.bacc as bacc
    import concourse.mybir as mybir
    import concourse.tile as tile

    f32 = mybir.dt.float32
    f16 = mybir.dt.float16
    AF = mybir.ActivationFunctionType
    OP = mybir.AluOpType

    nc = bacc.Bacc(None, target_bir_lowering=False)

    # all host-packed: per-partition contiguous per chunk / per e-tile
    xIn = nc.dram_tensor("xIn", [128, T * KT], f16, kind="ExternalInput")
    wzIn = nc.dram_tensor("wzIn", [128, ET * KT * 128], f16, kind="ExternalInput")
    whIn = nc.dram_tensor("whIn", [128, ET * KT * 128], f16, kind="ExternalInput")
    h0g = nc.dram_tensor("h0g", [128, ET], f32, kind="ExternalInput")
    hOut = nc.dram_tensor("hOut", [128, T * ET], f32, kind="ExternalOutput")

    with tile.TileContext(nc) as tc:
        with (
            tc.tile_pool(name="wpool", bufs=1) as wpool,
            tc.tile_pool(name="xpool", bufs=2) as xpool,
            tc.tile_pool(name="work", bufs=2) as work,
            tc.tile_pool(name="hpool", bufs=2) as hpool,
            tc.tile_pool(name="psum", bufs=1, space=bass.MemorySpace.PSUM) as psum,
        ):
            wz_sb = wpool.tile([128, ET, KT, 128], f16, tag="wz")
            wh_sb = wpool.tile([128, ET, KT, 128], f16, tag="wh")
            h0_sb = wpool.tile([128, ET], f32, tag="h0")
            warm = wpool.tile([128, 512], f16, tag="warm")
            nc.gpsimd.memset(warm[:], 0.0)

            sizes = SIZES
            offs = [sum(sizes[:i]) for i in range(len(sizes))]
            EB = KT * 128  # weight elems per partition per e-tile

            # startup loads: wz per e-tile on sync (first matmul group only
            # needs e=0), x0 in 2-kt pieces on scalar, wh on sync after wz
            # startup loads: wz per e-tile on sync (first matmul group only
            # needs e=0), x0 in 2-kt pieces on scalar, wh on sync after wz.
            # NOTE: more aggressive reorderings (wh interleave, weights on
            # scalar behind x0) measured neutral-to-worse: the Tile
            # scheduler paces DMA issue with bass_wait_until_ts from its own
            # cost model, so issue-order tricks fight the model, not HW.
            x0 = xpool.tile([128, KT, sizes[0]], f16, tag="x", name="x_0")

            def x0_load(eng, kp):
                eng.dma_start(
                    out=x0[:, 2 * kp:2 * kp + 2, :],
                    in_=xIn[:, 2 * kp * sizes[0]:(2 * kp + 2) * sizes[0]])

            # chunk-0 critical bytes in consumption order, split across both
            # HWDGE queues: sync carries wz_e0/wh_e0 + x0 p1/p3, scalar
            # carries x0 p0/p2; the weight remainder follows per e-tile in
            # (pk-e, pu-e) consumption order. Later x loads stay on sync
            # exactly as before (the Tile scheduler paces DMA issue from its
            # own cost-model sim; keeping the tail of the queue unchanged
            # keeps that pacing aligned).
            nc.sync.dma_start(out=wz_sb[:, 0, :, :], in_=wzIn[:, 0:EB])
            x0_load(nc.scalar, 0)
            nc.sync.dma_start(out=wh_sb[:, 0, :, :], in_=whIn[:, 0:EB])
            x0_load(nc.sync, 1)
            x0_load(nc.scalar, 2)
            x0_load(nc.sync, 3)
            for e in range(1, ET):
                nc.sync.dma_start(out=wz_sb[:, e, :, :],
                                  in_=wzIn[:, e * EB:(e + 1) * EB])
                nc.sync.dma_start(out=wh_sb[:, e, :, :],
                                  in_=whIn[:, e * EB:(e + 1) * EB])
            nc.gpsimd.dma_start(out=h0_sb[:], in_=h0g[:])

            h_prev = None
            prev_tc = 0
            for ci, (off, tc_) in enumerate(zip(offs, sizes)):
                tail = (ci >= len(sizes) - N_TAIL)
                if ci == 0:
                    x_sb = x0
                else:
                    x_sb = xpool.tile([128, KT, tc_], f16, tag="x",
                                      name=f"x_{ci}")
                    nc.sync.dma_start(
                        out=x_sb[:],
                        in_=xIn[:, off * KT:(off + tc_) * KT])

                pk = [psum.tile([128, tc_], f32, tag=f"pk{e}", name=f"pk{e}_{ci}")
                      for e in range(ET)]
                pu = [psum.tile([128, tc_], f32, tag=f"pu{e}", name=f"pu{e}_{ci}")
                      for e in range(ET)]

                if ci == 0:
                    # keep the PE busy (and its clock ramping) through the
                    # input DMA ramp
                    for _ in range(12):
                        nc.tensor.matmul(pk[0][:], warm[:, 0:128], warm[:, 0:tc_],
                                         start=True, stop=True)

                h = hpool.tile([128, ET, tc_], f32, tag="h", name=f"h_{ci}")
                for e in range(ET):
                    # tail chunk: pu before pk per e-tile, so t/g run
                    # during the pk matmuls and the post-chain is short
                    groups = ([(pu, wh_sb), (pk, wz_sb)] if tail
                              else [(pk, wz_sb), (pu, wh_sb)])
                    for dst, wsb in groups:
                        for kt in range(KT):
                            nc.tensor.matmul(dst[e][:], wsb[:, e, kt, :],
                                             x_sb[:, kt, :],
                                             start=(kt == 0), stop=(kt == KT - 1))
                for e in range(ET):
                    c = work.tile([128, tc_], f32, tag=f"c{e}", name=f"c{e}_{ci}")
                    t = work.tile([128, tc_], f16, tag=f"t{e}", name=f"t{e}_{ci}")
                    g = work.tile([128, tc_], f16, tag=f"g{e}", name=f"g{e}_{ci}")
                    vt = work.tile([128, tc_], f16, tag=f"v{e}", name=f"v{e}_{ci}")

                    # t issued before c: ACT runs in order, and in the tail
                    # chunk pu-e completes well before pk-e, so t (and g)
                    # finish during the pk matmuls instead of serializing
                    # behind c on the post-matmul critical path
                    nc.scalar.activation(t[:], pu[e][:], AF.Sigmoid, scale=1.0)
                    nc.scalar.activation(c[:], pk[e][:], AF.Sigmoid, scale=-1.0)
                    # g = max(u + 0.5, sigmoid(u)) == relu(u)+sigmoid(min(u,0))
                    nc.vector.scalar_tensor_tensor(g[:], pu[e][:], 0.5, t[:],
                                                   op0=OP.add, op1=OP.max)
                    # vt = (c-1)*g = -z*g, fused so no separate z op; the
                    # scan then uses op1=subtract (scan speed is ~2.6ns/elem
                    # regardless of ops/dtypes - measured)
                    nc.vector.scalar_tensor_tensor(vt[:], c[:], 1.0, g[:],
                                                   op0=OP.subtract, op1=OP.mult)
                    init = (h0_sb[:, e:e + 1] if ci == 0
                            else h_prev[:, e, prev_tc - 1:prev_tc])
                    nc.vector.tensor_tensor_scan(h[:, e, :], c[:], vt[:], init,
                                                 op0=OP.mult, op1=OP.subtract)
                    if tail:
                        eng = nc.sync if e % 2 == 0 else nc.scalar
                        eng.dma_start(
                            out=hOut[:, off * ET + e * tc_:off * ET + (e + 1) * tc_],
                            in_=h[:, e, :])
                h_prev = h
                prev_tc = tc_
                if not tail:
                    nc.scalar.dma_start(
                        out=hOut[:, off * ET:(off + tc_) * ET], in_=h[:])

    nc.compile()
    return nc


def _get_nc():
    if "nc" not in _CACHED:
        _CACHED["nc"] = _build_nc()
    return _CACHED["nc"]


def kernel(x, h0, W_h, W_z, _trace=False):
    global LAST_RESULT
    from concourse import bass_utils

    x = np.asarray(x, np.float32)
    h0 = np.asarray(h0, np.float32)
    W_h = np.asarray(W_h, np.float32)
    W_z = np.asarray(W_z, np.float32)

    # host-side prep: transposes + fp16 casts + initial state g(h0)
    gh0 = np.where(h0 >= 0, h0 + np.float32(0.5),
                   1.0 / (1.0 + np.exp(-h0))).astype(np.float32)  # [B,1,D]
    WzT = np.ascontiguousarray(W_z.T).astype(np.float16)  # [D, D]
    WhT = np.ascontiguousarray(W_h.T).astype(np.float16)

    offs = [sum(SIZES[:i]) for i in range(len(SIZES))]

    def pack_w(WT, esl):
        # [D, EC] -> [128, ET*KT*128], per-partition [ET, KT, 128]
        w = WT[:, esl].reshape(KT, 128, ET, 128)   # [kt, p, e, m]
        w = w.transpose(1, 2, 0, 3)                # [p, e, kt, m]
        return np.ascontiguousarray(w.reshape(128, ET * KT * 128))

    def pack_x(xTb):
        # [D, T] -> [128, T*KT]; chunk ci cols [off*KT,(off+tc)*KT) laid
        # out [KT, tc] per partition
        xk = xTb.reshape(KT, 128, T)
        parts = [xk[:, :, o:o + tcn].transpose(1, 0, 2).reshape(128, KT * tcn)
                 for o, tcn in zip(offs, SIZES)]
        return np.ascontiguousarray(np.concatenate(parts, axis=1))

    in_maps = []
    for b in range(B):
        xTb = np.ascontiguousarray(x[b].T).astype(np.float16)  # [D, T]
        xP = pack_x(xTb)
        for eh in range(2):
            esl = slice(eh * EC, (eh + 1) * EC)
            h0c = np.ascontiguousarray(
                gh0[b, 0, esl].reshape(ET, 128).T)  # [128, ET]
            in_maps.append({
                "xIn": xP,
                "wzIn": pack_w(WzT, esl),
                "whIn": pack_w(WhT, esl),
                "h0g": h0c,
            })

    nc = _get_nc()
    try:
        res = bass_utils.run_bass_kernel_spmd(
            nc, in_maps, core_ids=list(range(8)), trace=_trace,
        )
    except Exception:
        # transient NRT_EXEC_UNIT_UNRECOVERABLE has been observed on a
        # first execution; one retry has always succeeded
        res = bass_utils.run_bass_kernel_spmd(
            nc, in_maps, core_ids=list(range(8)), trace=_trace,
        )
    LAST_RESULT = res

    out = np.empty((B, T, D), np.float32)
    for b in range(B):
        for eh in range(2):
            core = b * 2 + eh
            arr = res.results[core]["hOut"]  # [128, T*ET] f32
            for o, tcn in zip(offs, SIZES):
                blk = arr[:, o * ET:(o + tcn) * ET].reshape(128, ET, tcn)
                out[b, o:o + tcn, eh * EC:(eh + 1) * EC] = (
                    blk.transpose(2, 1, 0).reshape(tcn, EC))
    return out


# revision 26
# speedup vs baseline: 1.0283x; 1.0283x over previous
"""MinGRU Trainium2 kernel.

Math (linear-space reformulation of the reference's log-space scan; all
quantities are positive so this is numerically safe):
    k = x @ W_z.T ; u = x @ W_h.T
    c_t = sigmoid(-k_t)                  # decay coeff (1 - z_t)
    g_t = max(sigmoid(u_t), u_t + 0.5)   # == relu(u) + sigmoid(min(u,0))
    vt_t = (c_t - 1) * g_t               # = -(z_t * g_t)
    h_t = c_t * h_{t-1} - vt_t           # hardware tensor_tensor_scan (mult, subtract)
    h_0 = g(h0)

Sharding: 8 cores = 4 batches x 2 halves of d_model output channels.
Each core: x.T [1024,4096] fp16, weight slices W.T[:, half] [1024,512]
fp16, out h.T [512,4096] f32.  fp16 upconverts exactly into the PE's
internal format and 2-byte stationary weights take the fast-weight-load
path so LDWEIGHTS (~97ns) hides under the previous matmul.

DMA layouts are host-packed so chunk transfers are contiguous per SBUF
partition (128 descriptors of 8-16KB), cutting DMA descriptor processing
~4x.  Weights are packed per e-tile so the first matmul group only waits
on 1/4 of W_z.  The scan keeps fp32 output (fp16 scan output hits a DVE
slow path measured at 3.5x).  Issue queues: sync carries wz/wh + x chunk
loads, scalar carries x0 + h writes; tail e-tile writes alternate
sync/scalar.  Dummy matmuls at t=0 (gated only on a gpsimd memset) keep
the PE busy/ramping through the input DMA ramp.
"""

import numpy as np

B, T, D = 4, 4096, 1024
EC = 512            # output channels per core
ET = EC // 128      # 4 e-tiles per core
KT = D // 128       # 8 k-tiles
SIZES = [256] + [512] * 7 + [256]   # time chunks (sum = T)
N_TAIL = 1                          # last chunk uses the low-latency tail path

_CACHED = {}
LAST_RESULT = None


def _build_nc():
    import concourse.bass as bass
    import concourse.bacc as bacc
    import concourse.mybir as mybir
    import concourse.tile as tile

    f32 = mybir.dt.float32
    f16 = mybir.dt.float16
    AF = mybir.ActivationFunctionType
    OP = mybir.AluOpType

    nc = bacc.Bacc(None, target_bir_lowering=False)

    # all host-packed: per-partition contiguous per chunk / per e-tile
    xIn = nc.dram_tensor("xIn", [128, T * KT], f16, kind="ExternalInput")
    wzIn = nc.dram_tensor("wzIn", [128, ET * KT * 128], f16, kind="ExternalInput")
    whIn = nc.dram_tensor("whIn", [128, ET * KT * 128], f16, kind="ExternalInput")
    h0g = nc.dram_tensor("h0g", [128, ET], f32, kind="ExternalInput")
    hOut = nc.dram_tensor("hOut", [128, T * ET], f32, kind="ExternalOutput")

    with tile.TileContext(nc) as tc:
        with (
            tc.tile_pool(name="wpool", bufs=1) as wpool,
            tc.tile_pool(name="xpool", bufs=2) as xpool,
            tc.tile_pool(name="work", bufs=2) as work,
            tc.tile_pool(name="hpool", bufs=2) as hpool,
            tc.tile_pool(name="psum", bufs=1, space=bass.MemorySpace.PSUM) as psum,
        ):
            wz_sb = wpool.tile([128, ET, KT, 128], f16, tag="wz")
            wh_sb = wpool.tile([128, ET, KT, 128], f16, tag="wh")
            h0_sb = wpool.tile([128, ET], f32, tag="h0")
            warm = wpool.tile([128, 512], f16, tag="warm")
            nc.gpsimd.memset(warm[:], 0.0)

            sizes = SIZES
            offs = [sum(sizes[:i]) for i in range(len(sizes))]
            EB = KT * 128  # weight elems per partition per e-tile

            # startup loads: wz per e-tile on sync (first matmul group only
            # needs e=0), x0 in 2-kt pieces on scalar, wh on sync after wz
            # startup loads: wz per e-tile on sync (first matmul group only
            # needs e=0), x0 in 2-kt pieces on scalar, wh on sync after wz.
            # NOTE: more aggressive reorderings (wh interleave, weights on
            # scalar behind x0) measured neutral-to-worse: the Tile
            # scheduler paces DMA issue with bass_wait_until_ts from its own
            # cost model, so issue-order tricks fight the model, not HW.
            x0 = xpool.tile([128, KT, sizes[0]], f16, tag="x", name="x_0")

            def x0_load(eng, kp):
                eng.dma_start(
                    out=x0[:, 2 * kp:2 * kp + 2, :],
                    in_=xIn[:, 2 * kp * sizes[0]:(2 * kp + 2) * sizes[0]])

            # chunk-0 critical bytes in consumption order, split across both
            # HWDGE queues: sync carries wz_e0/wh_e0 + x0 p1/p3, scalar
            # carries x0 p0/p2; the weight remainder follows per e-tile in
            # (pk-e, pu-e) consumption order. Later x loads stay on sync
            # exactly as before (the Tile scheduler paces DMA issue from its
            # own cost-model sim; keeping the tail of the queue unchanged
            # keeps that pacing aligned).
            nc.sync.dma_start(out=wz_sb[:, 0, :, :], in_=wzIn[:, 0:EB])
            x0_load(nc.scalar, 0)
            nc.sync.dma_start(out=wh_sb[:, 0, :, :], in_=whIn[:, 0:EB])
            x0_load(nc.sync, 1)
            x0_load(nc.scalar, 2)
            x0_load(nc.sync, 3)
            for e in range(1, ET):
                nc.sync.dma_start(out=wz_sb[:, e, :, :],
                                  in_=wzIn[:, e * EB:(e + 1) * EB])
                nc.sync.dma_start(out=wh_sb[:, e, :, :],
                                  in_=whIn[:, e * EB:(e + 1) * EB])
            nc.gpsimd.dma_start(out=h0_sb[:], in_=h0g[:])

            h_prev = None
            prev_tc = 0
            for ci, (off, tc_) in enumerate(zip(offs, sizes)):
                tail = (ci >= len(sizes) - N_TAIL)
                if ci == 0:
                    x_sb = x0
                else:
                    x_sb = xpool.tile([128, KT, tc_], f16, tag="x",
                                      name=f"x_{ci}")
                    nc.sync.dma_start(
                        out=x_sb[:],
                        in_=xIn[:, off * KT:(off + tc_) * KT])

                pk = [psum.tile([128, tc_], f32, tag=f"pk{e}", name=f"pk{e}_{ci}")
                      for e in range(ET)]
                pu = [psum.tile([128, tc_], f32, tag=f"pu{e}", name=f"pu{e}_{ci}")
                      for e in range(ET)]

                if ci == 0:
                    # keep the PE busy through the WHOLE input DMA ramp
                    # (~14us): idle gaps reset the PE clock ramp, and traces
                    # show chunk-0 matmuls running at mid p-state (213ns vs
                    # 109ns per 256 cols) when warmups end before x0 is fully
                    # resident. A longer train costs nothing while the PE
                    # would otherwise stall, and chunk-0 then runs full-clock.
                    for _ in range(40):
                        nc.tensor.matmul(pk[0][:], warm[:, 0:128], warm[:, 0:tc_],
                                         start=True, stop=True)

                h = hpool.tile([128, ET, tc_], f32, tag="h", name=f"h_{ci}")
                for e in range(ET):
                    # tail chunk: pu before pk per e-tile, so t/g run
                    # during the pk matmuls and the post-chain is short
                    groups = ([(pu, wh_sb), (pk, wz_sb)] if tail
                              else [(pk, wz_sb), (pu, wh_sb)])
                    for dst, wsb in groups:
                        for kt in range(KT):
                            nc.tensor.matmul(dst[e][:], wsb[:, e, kt, :],
                                             x_sb[:, kt, :],
                                             start=(kt == 0), stop=(kt == KT - 1))
                for e in range(ET):
                    c = work.tile([128, tc_], f32, tag=f"c{e}", name=f"c{e}_{ci}")
                    t = work.tile([128, tc_], f16, tag=f"t{e}", name=f"t{e}_{ci}")
                    g = work.tile([128, tc_], f16, tag=f"g{e}", name=f"g{e}_{ci}")
                    vt = work.tile([128, tc_], f16, tag=f"v{e}", name=f"v{e}_{ci}")

                    # t issued before c: ACT runs in order, and in the tail
                    # chunk pu-e completes well before pk-e, so t (and g)
                    # finish during the pk matmuls instead of serializing
                    # behind c on the post-matmul critical path
                    nc.scalar.activation(t[:], pu[e][:], AF.Sigmoid, scale=1.0)
                    nc.scalar.activation(c[:], pk[e][:], AF.Sigmoid, scale=-1.0)
                    # g = max(u + 0.5, sigmoid(u)) == relu(u)+sigmoid(min(u,0))
                    nc.vector.scalar_tensor_tensor(g[:], pu[e][:], 0.5, t[:],
                                                   op0=OP.add, op1=OP.max)
                    # vt = (c-1)*g = -z*g, fused so no separate z op; the
                    # scan then uses op1=subtract (scan speed is ~2.6ns/elem
                    # regardless of ops/dtypes - measured)
                    nc.vector.scalar_tensor_tensor(vt[:], c[:], 1.0, g[:],
                                                   op0=OP.subtract, op1=OP.mult)
                    init = (h0_sb[:, e:e + 1] if ci == 0
                            else h_prev[:, e, prev_tc - 1:prev_tc])
                    nc.vector.tensor_tensor_scan(h[:, e, :], c[:], vt[:], init,
                                                 op0=OP.mult, op1=OP.subtract)
                    if tail:
                        eng = nc.sync if e % 2 == 0 else nc.scalar
                        eng.dma_start(
                            out=hOut[:, off * ET + e * tc_:off * ET + (e + 1) * tc_],
                            in_=h[:, e, :])
                h_prev = h
                prev_tc = tc_
                if not tail:
                    nc.scalar.dma_start(
                        out=hOut[:, off * ET:(off + tc_) * ET], in_=h[:])

    nc.compile()
    return nc


def _get_nc():
    if "nc" not in _CACHED:
        _CACHED["nc"] = _build_nc()
    return _CACHED["nc"]


def kernel(x, h0, W_h, W_z, _trace=False):
    global LAST_RESULT
    from concourse import bass_utils

    x = np.asarray(x, np.float32)
    h0 = np.asarray(h0, np.float32)
    W_h = np.asarray(W_h, np.float32)
    W_z = np.asarray(W_z, np.float32)

    # host-side prep: transposes + fp16 casts + initial state g(h0)
    gh0 = np.where(h0 >= 0, h0 + np.float32(0.5),
                   1.0 / (1.0 + np.exp(-h0))).astype(np.float32)  # [B,1,D]
    WzT = np.ascontiguousarray(W_z.T).astype(np.float16)  # [D, D]
    WhT = np.ascontiguousarray(W_h.T).astype(np.float16)

    offs = [sum(SIZES[:i]) for i in range(len(SIZES))]

    def pack_w(WT, esl):
        # [D, EC] -> [128, ET*KT*128], per-partition [ET, KT, 128]
        w = WT[:, esl].reshape(KT, 128, ET, 128)   # [kt, p, e, m]
        w = w.transpose(1, 2, 0, 3)                # [p, e, kt, m]
        return np.ascontiguousarray(w.reshape(128, ET * KT * 128))

    def pack_x(xTb):
        # [D, T] -> [128, T*KT]; chunk ci cols [off*KT,(off+tc)*KT) laid
        # out [KT, tc] per partition
        xk = xTb.reshape(KT, 128, T)
        parts = [xk[:, :, o:o + tcn].transpose(1, 0, 2).reshape(128, KT * tcn)
                 for o, tcn in zip(offs, SIZES)]
        return np.ascontiguousarray(np.concatenate(parts, axis=1))

    in_maps = []
    for b in range(B):
        xTb = np.ascontiguousarray(x[b].T).astype(np.float16)  # [D, T]
        xP = pack_x(xTb)
        for eh in range(2):
            esl = slice(eh * EC, (eh + 1) * EC)
            h0c = np.ascontiguousarray(
                gh0[b, 0, esl].reshape(ET, 128).T)  # [128, ET]
            in_maps.append({
                "xIn": xP,
                "wzIn": pack_w(WzT, esl),
                "whIn": pack_w(WhT, esl),
                "h0g": h0c,
            })

    nc = _get_nc()
    try:
        res = bass_utils.run_bass_kernel_spmd(
            nc, in_maps, core_ids=list(range(8)), trace=_trace,
        )
    except Exception:
        # transient NRT_EXEC_UNIT_UNRECOVERABLE has been observed on a
        # first execution; one retry has always succeeded
        res = bass_utils.run_bass_kernel_spmd(
            nc, in_maps, core_ids=list(range(8)), trace=_trace,
        )
    LAST_RESULT = res

    out = np.empty((B, T, D), np.float32)
    for b in range(B):
        for eh in range(2):
            core = b * 2 + eh
            arr = res.results[core]["hOut"]  # [128, T*ET] f32
            for o, tcn in zip(offs, SIZES):
                blk = arr[:, o * ET:(o + tcn) * ET].reshape(128, ET, tcn)
                out[b, o:o + tcn, eh * EC:(eh + 1) * EC] = (
                    blk.transpose(2, 1, 0).reshape(tcn, EC))
    return out
